# revision 24
# baseline (speedup 1.0000x reference)
"""Bahdanau attention kernel for 8 Trainium2 NeuronCores.

Computation (per batch row b):
    scores[t] = Wo . tanh(We @ enc[t,b] + (mean_L(hidden) @ Wh.T + bh + be))
    out[b]    = masked_softmax(scores, enc_len[b])

Sharding: ragged-aware data parallelism. Each batch row only needs its
first ceil(enc_len/512) 512-token chunks (the masked softmax zeroes the
rest), so the host bin-packs whole batch rows onto the 8 cores by their
valid-chunk counts and ships, per core, a dense [NCK, 512, H] array of
just the valid chunks. The device program is fully static: NCK identical
work items per core (compiled per NCK and cached).

Per item: encoder states are cast fp32->bf16 inside the SWDGE load DMA,
transposed to [h, t] layout by the DMA xbar (one transpose-DMA per
chunk), contracted against We^T as bf16 PE matmuls with fp32 PSUM
accumulation, passed through one fused ScalarE tanh(x + bias) that reads
PSUM directly, and reduced against Wo with a K=o matmul into [1, 512]
chunk scores. The masked softmax runs item-parallel across partitions;
per-batch-row exp-sums (rows may own 2..4 items) are formed with tiny
host-built indicator matmuls, so raggedness never touches device control
flow.
"""

import math

import numpy as np
import ml_dtypes

import concourse.bass as bass
import concourse.mybir as mybir
import concourse.tile as tile
from concourse.bass_utils import run_bass_kernel_spmd

BF16 = mybir.dt.bfloat16
F32 = mybir.dt.float32

L, B, T, H = 2, 64, 2048, 512
NCORE = 8
BL = B // NCORE  # batch rows per core
CHUNK = 512
MAXROWS = 16
NEG = -1e30


def _split_multi_waits(nc):
    # This walrus build accepts only one sync-wait per instruction. Tile's
    # scheduler attaches one wait per outstanding proc (the exit drain can
    # carry many), so hoist extra waits onto single-wait NOP carriers
    # inserted just before the instruction on the same engine.
    for fn in nc.m.functions:
        for blk in fn.blocks:
            out = []
            changed = False
            for inst in blk.instructions:
                si = inst.sync_info
                waits = list(si.on_wait) if si is not None else []
                if len(waits) > 1:
                    changed = True
                    for k, w in enumerate(waits):
                        nop = mybir.InstNoOp(name=f"{inst.name}-wc{k}", ins=[], outs=[])
                        nop.engine = inst.engine
                        nop.sync_info = mybir.SyncInfo(on_wait=[w], on_update=[])
                        out.append(nop)
                    inst.sync_info = mybir.SyncInfo(
                        on_wait=[], on_update=list(si.on_update)
                    )
                out.append(inst)
            if changed:
                blk.instructions = out
    return nc


def _build_program(nck):
    nc = bass.Bass()
    enc = nc.dram_tensor("enc", [nck, CHUNK, H], BF16, kind="ExternalInput")
    wet = nc.dram_tensor("wet", [4, 128, H], BF16, kind="ExternalInput")
    wo = nc.dram_tensor("wo", [128, 4], BF16, kind="ExternalInput")
    cvi = nc.dram_tensor("cvi", [128, nck * 4], F32, kind="ExternalInput")
    mneg = nc.dram_tensor("mneg", [nck, CHUNK], F32, kind="ExternalInput")
    ind1 = nc.dram_tensor("ind1", [nck, MAXROWS], F32, kind="ExternalInput")
    ind2 = nc.dram_tensor("ind2", [MAXROWS, nck], F32, kind="ExternalInput")
    usum = nc.dram_tensor("usum", [MAXROWS, 1], F32, kind="ExternalInput")
    wout = nc.dram_tensor("wout", [nck, CHUNK], F32, kind="ExternalOutput")
    sc_dram = nc.dram_tensor("sc_scratch", [nck, CHUNK], F32)

    Tanh = mybir.ActivationFunctionType.Tanh
    Exp = mybir.ActivationFunctionType.Exp
    Copy = mybir.ActivationFunctionType.Copy

    with tile.TileContext(nc) as tc:
        with (
            tc.tile_pool(name="consts", bufs=1) as consts,
            tc.tile_pool(name="etp", bufs=6) as etp,
            tc.tile_pool(name="resp", bufs=8) as resp,
            tc.tile_pool(name="soft", bufs=1) as soft,
            tc.tile_pool(name="mmp", bufs=4, space="PSUM") as mmp,
            tc.tile_pool(name="scp", bufs=2, space="PSUM") as scp,
            tc.tile_pool(name="sfp", bufs=1, space="PSUM") as sfp,
        ):
            wet_sb = consts.tile([128, 4, H], BF16)
            nc.sync.dma_start(out=wet_sb, in_=wet[:].rearrange("c p o -> p c o"))
            wo_sb = consts.tile([128, 4], BF16)
            nc.sync.dma_start(out=wo_sb, in_=wo[:])
            cvi_sb = consts.tile([128, nck * 4], F32)
            nc.sync.dma_start(out=cvi_sb, in_=cvi[:])
            mneg_sb = consts.tile([128, CHUNK], F32)
            nc.sync.dma_start(out=mneg_sb[0:nck, :], in_=mneg[:])
            ind1_sb = consts.tile([128, MAXROWS], F32)
            nc.sync.dma_start(out=ind1_sb[0:nck, :], in_=ind1[:])
            ind2_sb = consts.tile([MAXROWS, nck], F32)
            nc.sync.dma_start(out=ind2_sb, in_=ind2[:])
            usum_sb = consts.tile([MAXROWS, 1], F32)
            nc.sync.dma_start(out=usum_sb, in_=usum[:])
            # per-item scores staged on partition 0 (compute engines cannot
            # write APs at unaligned base partitions), then bounced through
            # DRAM into [nck, 512] item-parallel layout for the softmax
            scores_flat = consts.tile([1, nck * CHUNK], F32)

            for i in range(nck):
                # transpose-load the chunk straight from DRAM via the xbar:
                # encT[p, hc, n*128+t] = enc[i, n*128+t, hc*128+p]
                encT = etp.tile([128, 4, CHUNK], BF16)
                for n in range(4):
                    nc.sync.dma_start_transpose(
                        encT[:, :, n * 128 : (n + 1) * 128],
                        enc[i, n * 128 : (n + 1) * 128, :],
                    )
                res_tiles = []
                for oc in range(4):
                    matt = mmp.tile([128, CHUNK], F32)
                    for hc in range(4):
                        nc.tensor.matmul(
                            matt,
                            wet_sb[:, hc, oc * 128 : (oc + 1) * 128],
                            encT[:, hc, :],
                            start=(hc == 0),
                            stop=(hc == 3),
                        )
                    res = resp.tile([128, CHUNK], BF16)
                    nc.scalar.activation(
                        res, matt, Tanh, bias=cvi_sb[:, i * 4 + oc : i * 4 + oc + 1]
                    )
                    res_tiles.append(res)
                score = scp.tile([1, CHUNK], F32)
                for oc in range(4):
                    nc.tensor.matmul(
                        score,
                        wo_sb[:, oc : oc + 1],
                        res_tiles[oc],
                        start=(oc == 0),
                        stop=(oc == 3),
                    )
                nc.vector.tensor_copy(
                    scores_flat[0:1, i * CHUNK : (i + 1) * CHUNK], score
                )

            nc.sync.dma_start(
                out=sc_dram[:], in_=scores_flat.rearrange("p (i t) -> p i t", i=nck)
            )
            scores_sb = soft.tile([128, CHUNK], F32)
            nc.sync.dma_start(out=scores_sb[0:nck, :], in_=sc_dram[:])

            # masked softmax, item-parallel across partitions; per-batch-row
            # exp sums via host-built indicator matmuls
            sm = soft.tile([128, CHUNK], F32)
            nc.vector.tensor_add(sm[0:nck], scores_sb[0:nck, :], mneg_sb[0:nck, :])
            ex = soft.tile([128, CHUNK], F32)
            partials = soft.tile([128, 1], F32)
            nc.scalar.activation(
                ex[0:nck], sm[0:nck], Exp, accum_out=partials[0:nck]
            )
            sums_ps = sfp.tile([MAXROWS, 1], F32)
            nc.tensor.matmul(
                sums_ps, ind1_sb[0:nck, :], partials[0:nck], start=True, stop=True
            )
            sums_sb = soft.tile([MAXROWS, 1], F32)
            nc.vector.tensor_add(sums_sb, sums_ps, usum_sb)
            rec = soft.tile([MAXROWS, 1], F32)
            nc.vector.reciprocal(rec, sums_sb)
            reci_ps = sfp.tile([128, 1], F32)
            nc.tensor.matmul(
                reci_ps[0:nck], ind2_sb, rec, start=True, stop=True
            )
            reci = soft.tile([128, 1], F32)
            nc.vector.tensor_copy(reci[0:nck], reci_ps[0:nck])
            wt = soft.tile([128, CHUNK], F32)
            nc.vector.tensor_scalar_mul(wt[0:nck], ex[0:nck], reci[0:nck])
            nc.sync.dma_start(out=wout[:], in_=wt[0:nck, :])

    return _split_multi_waits(nc)


_PROGRAMS = {}


def _program(nck):
    if nck not in _PROGRAMS:
        _PROGRAMS[nck] = _build_program(nck)
    return _PROGRAMS[nck]


def kernel(hidden, encoder_output, enc_len, Wh, bh, We, be, Wo, bo):
    hidden = np.asarray(hidden, dtype=np.float32)
    encoder_output = np.asarray(encoder_output, dtype=np.float32)
    enc_len = np.asarray(enc_len, dtype=np.int32)
    Wh = np.asarray(Wh, dtype=np.float32)
    bh = np.asarray(bh, dtype=np.float32)
    We = np.asarray(We, dtype=np.float32)
    be = np.asarray(be, dtype=np.float32)
    Wo = np.asarray(Wo, dtype=np.float32)
    bo = np.asarray(bo, dtype=np.float32)

    # small host-side prep: bias vector c = mean_L(hidden) @ Wh.T + bh + be
    h = hidden.mean(axis=0, dtype=np.float64)  # [B, H]
    c = (h @ Wh.T.astype(np.float64) + bh + be).astype(np.float32)  # [B, H]

    wet_np = np.ascontiguousarray(We.T).reshape(4, 128, H).astype(ml_dtypes.bfloat16)
    wo_np = np.ascontiguousarray(Wo.reshape(4, 128).T).astype(ml_dtypes.bfloat16)

    # valid 512-token chunks per batch row; bin-pack rows onto cores
    nck_b = np.minimum(np.maximum((enc_len + CHUNK - 1) // CHUNK, 1), T // CHUNK)
    order = np.argsort(-nck_b, kind="stable")
    core_rows = [[] for _ in range(NCORE)]
    core_load = [0] * NCORE
    for b in order:
        candidates = [k for k in range(NCORE) if len(core_rows[k]) < MAXROWS]
        cmin = min(candidates, key=lambda k: core_load[k])
        core_rows[cmin].append(int(b))
        core_load[cmin] += int(nck_b[b])
    nck = max(core_load)

    tvec = np.arange(T)
    in_maps = []
    items_per_core = []
    for core in range(NCORE):
        rows = core_rows[core]
        items = []  # (b, ck)
        for b in rows:
            for ck in range(int(nck_b[b])):
                items.append((b, ck))
        real_n = len(items)
        while len(items) < nck:
            items.append(items[-1])  # padding item (excluded from indicators)
        items_per_core.append((rows, items, real_n))

        enc_c = np.empty((nck, CHUNK, H), dtype=ml_dtypes.bfloat16)
        cvi = np.zeros((128, nck * 4), dtype=np.float32)
        mneg_c = np.full((nck, CHUNK), NEG, dtype=np.float32)
        ind1 = np.zeros((nck, MAXROWS), dtype=np.float32)
        ind2 = np.zeros((MAXROWS, nck), dtype=np.float32)
        usum_c = np.zeros((MAXROWS, 1), dtype=np.float32)
        usum_c[len(rows):, 0] = 1.0  # keep unused row slots finite (1/1)
        for i, (b, ck) in enumerate(items):
            enc_c[i] = encoder_output[ck * CHUNK : (ck + 1) * CHUNK, b, :]
            cvi[:, i * 4 : (i + 1) * 4] = c[b].reshape(4, 128).T
            if i < real_n:
                mneg_c[i] = np.where(
                    tvec[ck * CHUNK : (ck + 1) * CHUNK] < enc_len[b], 0.0, NEG
                )
                lb = rows.index(b)
                ind1[i, lb] = 1.0
                ind2[lb, i] = 1.0
        in_maps.append(
            {
                "enc": enc_c,
                "wet": wet_np,
                "wo": wo_np,
                "cvi": cvi,
                "mneg": mneg_c,
                "ind1": ind1,
                "ind2": ind2,
                "usum": usum_c,
            }
        )

    nc = _program(nck)
    results = run_bass_kernel_spmd(nc, in_maps, list(range(NCORE))).results

    w = np.zeros((B, T), dtype=np.float32)
    for core in range(NCORE):
        rows, items, real_n = items_per_core[core]
        wc = results[core]["wout"]  # [nck, 512]
        for i in range(real_n):
            b, ck = items[i]
            w[b, ck * CHUNK : (ck + 1) * CHUNK] = wc[i]
    return w[:, :, None].astype(np.float32)


# revision 25
# speedup vs baseline: 1.1002x; 1.1002x over previous
"""Bahdanau attention kernel for 8 Trainium2 NeuronCores.

Computation (per batch row b):
    scores[t] = Wo . tanh(We @ enc[t,b] + (mean_L(hidden) @ Wh.T + bh + be))
    out[b]    = masked_softmax(scores, enc_len[b])

Sharding: ragged-aware data parallelism. Each batch row only needs its
first ceil(enc_len/512) 512-token chunks (the masked softmax zeroes the
rest), so the host bin-packs whole batch rows onto the 8 cores by their
valid-chunk counts and ships, per core, a dense [NCK, 512, H] array of
just the valid chunks. The device program is fully static: NCK identical
work items per core (compiled per NCK and cached).

Per item: encoder states are cast fp32->bf16 inside the SWDGE load DMA,
transposed to [h, t] layout by the DMA xbar (one transpose-DMA per
chunk), contracted against We^T as bf16 PE matmuls with fp32 PSUM
accumulation, passed through one fused ScalarE tanh(x + bias) that reads
PSUM directly, and reduced against Wo with a K=o matmul into [1, 512]
chunk scores. The masked softmax runs item-parallel across partitions;
per-batch-row exp-sums (rows may own 2..4 items) are formed with tiny
host-built indicator matmuls, so raggedness never touches device control
flow.
"""

import math

import numpy as np
import ml_dtypes

import concourse.bass as bass
import concourse.mybir as mybir
import concourse.tile as tile
from concourse.bass_utils import run_bass_kernel_spmd

BF16 = mybir.dt.bfloat16
F32 = mybir.dt.float32

L, B, T, H = 2, 64, 2048, 512
NCORE = 8
BL = B // NCORE  # batch rows per core
CHUNK = 512
MAXROWS = 16
NEG = -1e30


def _split_multi_waits(nc):
    # This walrus build accepts only one sync-wait per instruction. Tile's
    # scheduler attaches one wait per outstanding proc (the exit drain can
    # carry many), so hoist extra waits onto single-wait NOP carriers
    # inserted just before the instruction on the same engine.
    for fn in nc.m.functions:
        for blk in fn.blocks:
            out = []
            changed = False
            for inst in blk.instructions:
                si = inst.sync_info
                waits = list(si.on_wait) if si is not None else []
                if len(waits) > 1:
                    changed = True
                    for k, w in enumerate(waits):
                        nop = mybir.InstNoOp(name=f"{inst.name}-wc{k}", ins=[], outs=[])
                        nop.engine = inst.engine
                        nop.sync_info = mybir.SyncInfo(on_wait=[w], on_update=[])
                        out.append(nop)
                    inst.sync_info = mybir.SyncInfo(
                        on_wait=[], on_update=list(si.on_update)
                    )
                out.append(inst)
            if changed:
                blk.instructions = out
    return nc


def _build_program(nck):
    nc = bass.Bass()
    enc = nc.dram_tensor("enc", [nck, CHUNK, H], BF16, kind="ExternalInput")
    wet = nc.dram_tensor("wet", [4, 128, H], BF16, kind="ExternalInput")
    wo = nc.dram_tensor("wo", [128, 4], BF16, kind="ExternalInput")
    cvi = nc.dram_tensor("cvi", [128, nck * 4], F32, kind="ExternalInput")
    mneg = nc.dram_tensor("mneg", [nck, CHUNK], F32, kind="ExternalInput")
    ind1 = nc.dram_tensor("ind1", [nck, MAXROWS], F32, kind="ExternalInput")
    ind2 = nc.dram_tensor("ind2", [MAXROWS, nck], F32, kind="ExternalInput")
    usum = nc.dram_tensor("usum", [MAXROWS, 1], F32, kind="ExternalInput")
    wout = nc.dram_tensor("wout", [nck, CHUNK], F32, kind="ExternalOutput")
    sc_dram = nc.dram_tensor("sc_scratch", [nck, CHUNK], F32)

    Tanh = mybir.ActivationFunctionType.Tanh
    Exp = mybir.ActivationFunctionType.Exp
    Copy = mybir.ActivationFunctionType.Copy

    with tile.TileContext(nc) as tc:
        with (
            tc.tile_pool(name="consts", bufs=1) as consts,
            tc.tile_pool(name="etp", bufs=8) as etp,
            tc.tile_pool(name="resp", bufs=8) as resp,
            tc.tile_pool(name="soft", bufs=1) as soft,
            tc.tile_pool(name="mmp", bufs=4, space="PSUM") as mmp,
            tc.tile_pool(name="scp", bufs=2, space="PSUM") as scp,
            tc.tile_pool(name="sfp", bufs=1, space="PSUM") as sfp,
        ):
            wet_sb = consts.tile([128, 4, H], BF16)
            nc.gpsimd.dma_start(out=wet_sb, in_=wet[:].rearrange("c p o -> p c o"))
            wo_sb = consts.tile([128, 4], BF16)
            nc.gpsimd.dma_start(out=wo_sb, in_=wo[:])
            cvi_sb = consts.tile([128, nck * 4], F32)
            nc.gpsimd.dma_start(out=cvi_sb, in_=cvi[:])
            mneg_sb = consts.tile([128, CHUNK], F32)
            nc.gpsimd.dma_start(out=mneg_sb[0:nck, :], in_=mneg[:])
            ind1_sb = consts.tile([128, MAXROWS], F32)
            nc.gpsimd.dma_start(out=ind1_sb[0:nck, :], in_=ind1[:])
            ind2_sb = consts.tile([MAXROWS, nck], F32)
            nc.gpsimd.dma_start(out=ind2_sb, in_=ind2[:])
            usum_sb = consts.tile([MAXROWS, 1], F32)
            nc.gpsimd.dma_start(out=usum_sb, in_=usum[:])
            # per-item scores staged on partition 0 (compute engines cannot
            # write APs at unaligned base partitions), then bounced through
            # DRAM into [nck, 512] item-parallel layout for the softmax
            scores_flat = consts.tile([1, nck * CHUNK], F32)

            for i in range(nck):
                # transpose-load the whole chunk straight from DRAM in ONE
                # xbar DMA: view the [512, 512] chunk as [128 rows, 2048]
                # (4 tokens per row), so out[p, c, r] = enc[i, 4r + c%4?, ...]:
                # with f = q*512 + h, c = q*4 + hc -> out[p, q*4+hc, r] =
                # enc[i, 4r+q, hc*128+p]. Token order inside N is (q, r),
                # i.e. token = 4r + q; the host permutes mask/output to match.
                encT = etp.tile([128, 16, 128], BF16)
                nc.sync.dma_start_transpose(
                    encT, enc[i].rearrange("(r x) h -> r (x h)", x=4)
                )
                encT_v = encT.rearrange("p (q c) t -> p c q t", c=4)
                res_tiles = []
                for oc in range(4):
                    matt = mmp.tile([128, CHUNK], F32)
                    for hc in range(4):
                        nc.tensor.matmul(
                            matt,
                            wet_sb[:, hc, oc * 128 : (oc + 1) * 128],
                            encT_v[:, hc, :, :],
                            start=(hc == 0),
                            stop=(hc == 3),
                        )
                    res = resp.tile([128, CHUNK], BF16)
                    nc.scalar.activation(
                        res, matt, Tanh, bias=cvi_sb[:, i * 4 + oc : i * 4 + oc + 1]
                    )
                    res_tiles.append(res)
                score = scp.tile([1, CHUNK], F32)
                for oc in range(4):
                    nc.tensor.matmul(
                        score,
                        wo_sb[:, oc : oc + 1],
                        res_tiles[oc],
                        start=(oc == 0),
                        stop=(oc == 3),
                    )
                nc.vector.tensor_copy(
                    scores_flat[0:1, i * CHUNK : (i + 1) * CHUNK], score
                )

            nc.sync.dma_start(
                out=sc_dram[:], in_=scores_flat.rearrange("p (i t) -> p i t", i=nck)
            )
            scores_sb = soft.tile([128, CHUNK], F32)
            nc.sync.dma_start(out=scores_sb[0:nck, :], in_=sc_dram[:])

            # masked softmax, item-parallel across partitions; per-batch-row
            # exp sums via host-built indicator matmuls
            sm = soft.tile([128, CHUNK], F32)
            nc.vector.tensor_add(sm[0:nck], scores_sb[0:nck, :], mneg_sb[0:nck, :])
            ex = soft.tile([128, CHUNK], F32)
            partials = soft.tile([128, 1], F32)
            nc.scalar.activation(
                ex[0:nck], sm[0:nck], Exp, accum_out=partials[0:nck]
            )
            sums_ps = sfp.tile([MAXROWS, 1], F32)
            nc.tensor.matmul(
                sums_ps, ind1_sb[0:nck, :], partials[0:nck], start=True, stop=True
            )
            sums_sb = soft.tile([MAXROWS, 1], F32)
            nc.vector.tensor_add(sums_sb, sums_ps, usum_sb)
            rec = soft.tile([MAXROWS, 1], F32)
            nc.vector.reciprocal(rec, sums_sb)
            reci_ps = sfp.tile([128, 1], F32)
            nc.tensor.matmul(
                reci_ps[0:nck], ind2_sb, rec, start=True, stop=True
            )
            reci = soft.tile([128, 1], F32)
            nc.vector.tensor_copy(reci[0:nck], reci_ps[0:nck])
            wt = soft.tile([128, CHUNK], F32)
            nc.vector.tensor_scalar_mul(wt[0:nck], ex[0:nck], reci[0:nck])
            nc.sync.dma_start(out=wout[:], in_=wt[0:nck, :])

    return _split_multi_waits(nc)


_PROGRAMS = {}


def _program(nck):
    if nck not in _PROGRAMS:
        _PROGRAMS[nck] = _build_program(nck)
    return _PROGRAMS[nck]


def kernel(hidden, encoder_output, enc_len, Wh, bh, We, be, Wo, bo):
    hidden = np.asarray(hidden, dtype=np.float32)
    encoder_output = np.asarray(encoder_output, dtype=np.float32)
    enc_len = np.asarray(enc_len, dtype=np.int32)
    Wh = np.asarray(Wh, dtype=np.float32)
    bh = np.asarray(bh, dtype=np.float32)
    We = np.asarray(We, dtype=np.float32)
    be = np.asarray(be, dtype=np.float32)
    Wo = np.asarray(Wo, dtype=np.float32)
    bo = np.asarray(bo, dtype=np.float32)

    # small host-side prep: bias vector c = mean_L(hidden) @ Wh.T + bh + be
    h = hidden.mean(axis=0, dtype=np.float64)  # [B, H]
    c = (h @ Wh.T.astype(np.float64) + bh + be).astype(np.float32)  # [B, H]

    wet_np = np.ascontiguousarray(We.T).reshape(4, 128, H).astype(ml_dtypes.bfloat16)
    wo_np = np.ascontiguousarray(Wo.reshape(4, 128).T).astype(ml_dtypes.bfloat16)

    # valid 512-token chunks per batch row; bin-pack rows onto cores
    nck_b = np.minimum(np.maximum((enc_len + CHUNK - 1) // CHUNK, 1), T // CHUNK)
    order = np.argsort(-nck_b, kind="stable")
    core_rows = [[] for _ in range(NCORE)]
    core_load = [0] * NCORE
    for b in order:
        candidates = [k for k in range(NCORE) if len(core_rows[k]) < MAXROWS]
        cmin = min(candidates, key=lambda k: core_load[k])
        core_rows[cmin].append(int(b))
        core_load[cmin] += int(nck_b[b])
    nck = max(core_load)

    tvec = np.arange(T)
    # N-position q*128+r holds token 4r+q within the chunk
    tok_of_pos = (4 * np.arange(128)[None, :] + np.arange(4)[:, None]).reshape(CHUNK)
    in_maps = []
    items_per_core = []
    for core in range(NCORE):
        rows = core_rows[core]
        items = []  # (b, ck)
        for b in rows:
            for ck in range(int(nck_b[b])):
                items.append((b, ck))
        real_n = len(items)
        while len(items) < nck:
            items.append(items[-1])  # padding item (excluded from indicators)
        items_per_core.append((rows, items, real_n))

        enc_c = np.empty((nck, CHUNK, H), dtype=ml_dtypes.bfloat16)
        cvi = np.zeros((128, nck * 4), dtype=np.float32)
        mneg_c = np.full((nck, CHUNK), NEG, dtype=np.float32)
        ind1 = np.zeros((nck, MAXROWS), dtype=np.float32)
        ind2 = np.zeros((MAXROWS, nck), dtype=np.float32)
        usum_c = np.zeros((MAXROWS, 1), dtype=np.float32)
        usum_c[len(rows):, 0] = 1.0  # keep unused row slots finite (1/1)
        for i, (b, ck) in enumerate(items):
            enc_c[i] = encoder_output[ck * CHUNK : (ck + 1) * CHUNK, b, :]
            cvi[:, i * 4 : (i + 1) * 4] = c[b].reshape(4, 128).T
            if i < real_n:
                mneg_c[i] = np.where(
                    (ck * CHUNK + tok_of_pos) < enc_len[b], 0.0, NEG
                )
                lb = rows.index(b)
                ind1[i, lb] = 1.0
                ind2[lb, i] = 1.0
        in_maps.append(
            {
                "enc": enc_c,
                "wet": wet_np,
                "wo": wo_np,
                "cvi": cvi,
                "mneg": mneg_c,
                "ind1": ind1,
                "ind2": ind2,
                "usum": usum_c,
            }
        )

    nc = _program(nck)
    results = run_bass_kernel_spmd(nc, in_maps, list(range(NCORE))).results

    w = np.zeros((B, T), dtype=np.float32)
    for core in range(NCORE):
        rows, items, real_n = items_per_core[core]
        wc = results[core]["wout"]  # [nck, 512]
        for i in range(real_n):
            b, ck = items[i]
            w[b, ck * CHUNK + tok_of_pos] = wc[i]
    return w[:, :, None].astype(np.float32)
